# revision 1
# baseline (speedup 1.0000x reference)
"""Trainium2 Bass kernel for a dense transformer block (RMSNorm -> causal MHA
-> residual -> RMSNorm -> SwiGLU FFN -> residual).

Sharding: data-parallel over tokens with a *strided* assignment -- core c
owns every token position == c (mod 8) of both batches (512 tokens/core),
weights replicated. The stride makes the causal chunk structure identical on
every core (no padding waste), so one SPMD program serves all cores; the
residual per-core causality lands in small per-core mask *data* for the
diagonal key chunks only. K/V are exchanged with one AllGather pair.

v2: qkv, out-proj, and the attention V-aggregation run in fp8(e4m3) using
DoubleRow matmuls (2 k-tiles per instruction, ~1.95x bf16 throughput,
HW-validated). The FFN stays bf16 (fp8 there costs ~3.3e-2 rel err, over the
2e-2 gate; compensation would erase the speed gain). Softmax denominators
accumulate on the PE via a fp8 DoubleRow ones-matmul (one instr per chunk
pair), the old [1,512] PSUM + slow DVE TensorReduce chain is gone, and the
causal mask is a post-exp 0/1 fp8 multiply on the e-tile.

Scale bookkeeping (all folded into host prep / PSUM-evacuation copies):
  xn8 = 4*xn,  w8 = 16*w  -> qkv psum = 64*true -> q,k fp8 at 16x true
  (copy scale 1/4); logits psum = 256*true -> exp scale 1/256
  v8 = 4*v (copy scale 1/16 from 64*true psum)
  e8 = exp(logit - 1.5)  (max logit ~6 -> e8 <= ~99 << 240; typical
  weights stay in fp8 normal range so short-window queries keep precision)
  otp = sum(e8 * v8) ; r8 = sum(e8) (same scale -> cancels; v8 scale 4
  makes ot8 = 4*attn_out exactly)
  ot8 = otp * (1/r8) = 4*attn_out  (exactly the fp8 input scale for out-proj)
  out-proj psum = 4*16*true -> residual-add copy scale 1/64
"""

import math
from dataclasses import dataclass
from contextlib import ExitStack

import numpy as np

import concourse.bacc as bacc
import concourse.mybir as mybir
import concourse.tile as tile
from concourse.bass_utils import run_bass_kernel_spmd

try:
    import ml_dtypes

    BF16 = ml_dtypes.bfloat16
    E4M3 = ml_dtypes.float8_e4m3
except ImportError:  # pragma: no cover
    import jax.numpy as jnp

    BF16 = jnp.bfloat16
    E4M3 = jnp.float8_e4m3

F32 = mybir.dt.float32
BF = mybir.dt.bfloat16
F8 = mybir.dt.float8e4
AF = mybir.ActivationFunctionType
DR = mybir.MatmulPerfMode.DoubleRow

SX = 4.0     # fp8 scale on activations (xn, v, attn-out)
SW = 16.0    # fp8 scale on weights
EBIAS = 1.5  # e8 = exp(logit - EBIAS); max ~e^4.6=99 << 240


@dataclass(frozen=True)
class Cfg:
    B: int = 2
    S: int = 2048
    D: int = 2048
    H: int = 16
    DFF: int = 8192
    n_cores: int = 8
    eps: float = 1e-6
    use_silu: bool = True   # ACT Silu table vs sigmoid+mul fallback
    fake_ag: bool = False   # replace AllGather with local DMA (TimelineSim)
    fp8_qkv: bool = True    # fp8 DoubleRow qkv projection
    fp8_out: bool = True    # fp8 DoubleRow out projection
    fp8_av: bool = True     # fp8 V transport + DR attention aggregation

    @property
    def Hd(self):
        return self.D // self.H

    @property
    def S_blk(self):
        return self.S // self.n_cores  # per-(core,batch) query block

    @property
    def T(self):
        return self.B * self.S_blk  # tokens per core

    @property
    def TB(self):
        return self.T // 128

    @property
    def DC(self):
        return self.D // 128

    @property
    def DP(self):
        return self.DC // 2  # k-tile pairs in a D contraction

    @property
    def SB2(self):
        return self.S_blk // 128

    @property
    def NP2(self):
        return self.n_cores // 2

    @property
    def FFB(self):
        return self.DFF // 128

    @property
    def CT(self):
        return max(1, self.D // 512)

    @property
    def CW(self):
        return min(512, self.D)

    @property
    def W2G(self):
        return 4  # w2 chunks per DMA group


FULL = Cfg()


def build_nc(cfg: Cfg):
    """Build the per-core Bass program (identical on all cores)."""
    assert cfg.Hd == 128 and cfg.S_blk % 128 == 0 and cfg.T % 128 == 0
    n, D, H, T, DC, DP = cfg.n_cores, cfg.D, cfg.H, cfg.T, cfg.DC, cfg.DP
    S_blk, TB, CT, CW, DFF, FFB, W2G = (
        cfg.S_blk, cfg.TB, cfg.CT, cfg.CW, cfg.DFF, cfg.FFB, cfg.W2G)
    SB2, NP2 = cfg.SB2, cfg.NP2
    assert FFB % W2G == 0 and n % 4 == 0 and T % 256 == 0 and T == 512

    nc = bacc.Bacc("TRN2", target_bir_lowering=False, debug=False,
                   num_devices=n)

    # ---- I/O (host pre-tiled layouts) ----
    x_io = nc.dram_tensor("x_c", [T, D], F32, kind="ExternalInput")
    # Q/K weight blocks for DoubleRow: [32, 128, DP, 2, 128] (K heads first)
    wqk_io = nc.dram_tensor("wqk8", [2 * H, 128, DP, 2, 128], F8,
                            kind="ExternalInput")
    # V weights as DR moving operand: [4 groups, 128, DP, 2, 512]
    wv_io = nc.dram_tensor("wv8", [D // 512, 128, DP, 2, 512], F8,
                           kind="ExternalInput")
    # out-proj: [4 groups, 128, H/2 head-pairs, 2, 512]
    wout_io = nc.dram_tensor("wout8", [CT, 128, H // 2, 2, CW], F8,
                             kind="ExternalInput")
    w13_io = nc.dram_tensor("w13_t", [FFB // 2, 128, DC, 512], BF,
                            kind="ExternalInput")
    # quarter-fp8 FFN-up weights: groups 0..7 (ff cols 0..4095 interleaved
    # w1/w3), DR layout [8, 128, 4 colblocks, DP, 2, 128]
    w13f8_io = nc.dram_tensor("w13f8", [8, 128, 4, DP, 2, 128], F8,
                              kind="ExternalInput")
    w2_io = nc.dram_tensor("w2_t", [FFB // W2G, 128, W2G, D], BF,
                           kind="ExternalInput")
    # 0/1 causal masks for diagonal chunks: [128, NP2, 2, 2, 128] fp8
    masks_io = nc.dram_tensor("masks", [128, NP2, 2, 2, 128], F8,
                              kind="ExternalInput")
    ident_io = nc.dram_tensor("ident", [128, 128], BF, kind="ExternalInput")
    ident8_io = nc.dram_tensor("ident8", [128, 128], F8, kind="ExternalInput")
    ones_io = nc.dram_tensor("ones128", [128, 128], BF, kind="ExternalInput")
    ones8_io = nc.dram_tensor("ones8", [128, 2, 128], F8, kind="ExternalInput")
    out_io = nc.dram_tensor("out_c", [T, D], F32, kind="ExternalOutput")

    with tile.TileContext(nc) as tc, ExitStack() as top:
        P = top.enter_context(tc.tile_pool(name="persist", bufs=1))
        consts = top.enter_context(tc.tile_pool(name="consts", bufs=1))

        ident = consts.tile([128, 128], BF, name="ident_sb")
        ident8 = consts.tile([128, 128], F8, name="ident8_sb")
        ones = consts.tile([128, 128], BF, name="ones_sb")
        ones8 = consts.tile([128, 2, 128], F8, name="ones8_sb")
        mask_sb = consts.tile([128, NP2, 2, 2, 128], F8, name="mask_sb")
        eps16_t = consts.tile([128, 1], F32, name="eps16_sb")
        eps_t = consts.tile([128, 1], F32, name="eps_sb")
        nebias_t = consts.tile([128, 1], F32, name="nebias_sb")
        nc.any.memset(eps16_t[:], cfg.eps / (SX * SX))
        nc.any.memset(eps_t[:], cfg.eps)
        nc.any.memset(nebias_t[:], -EBIAS)
        c64i = consts.tile([128, 1], F32, name="c64i_sb")
        c16i = consts.tile([128, 1], F32, name="c16i_sb")
        c4i = consts.tile([128, 1], F32, name="c4i_sb")
        nc.any.memset(c64i[:], 1.0 / (SX * SW))
        nc.any.memset(c16i[:], 1.0 / SW)
        nc.any.memset(c4i[:], 0.25)
        c4 = consts.tile([128, 1], F32, name="c4_sb")
        nc.any.memset(c4[:], 4.0)
        nc.sync.dma_start(ident8[:], ident8_io[:, :])
        nc.sync.dma_start(ident[:], ident_io[:, :])

        x1_t = [P.tile([128, D], F32, name=f"x1_{tb}") for tb in range(TB)]
        qt = P.tile([128, H, T], F8, name="qt")          # Q d-major fp8 (16x)
        ot8 = P.tile([128, H, T], F8, name="ot8")        # attn out d-major fp8
        xzt8 = P.tile([128, DC, T], F8, name="xzt8")     # xn^T fp8 (qkv in)
        xzt2 = P.tile([128, DC, T], BF, name="xzt2")     # z2^T bf16 (ffn in)
        xzt2f8 = P.tile([128, DC, T], F8, name="xzt2f8") # z2^T fp8 (4x)

        dram = top.enter_context(tc.tile_pool(name="dram", bufs=1,
                                              space="DRAM"))
        kt_c = dram.tile([D, T], F8, name="kt_contrib")
        v_c = dram.tile([T, D], F8, name="v_contrib")
        kt_g = dram.tile([n * D, T], F8, name="kt_gath", addr_space="Shared")
        v_g = dram.tile([n * T, D], F8, name="v_gath", addr_space="Shared")
        kt_g3 = kt_g[:].rearrange("(r p) t -> r p t", r=n)
        v_g4 = v_g[:].rearrange("(a b tk) d -> a b tk d", a=NP2, b=2)
        hh_d = dram.tile([DFF, T], BF, name="hh_d")      # swiglu spill

        # ================= phase 1: rmsnorm1 -> fp8 -> transpose ==========
        with tc.tile_pool(name="ph1", bufs=2) as ph1, \
             tc.tile_pool(name="ps_tp1", bufs=4, space="PSUM") as ps_tp1:
            for tb in range(TB):
                xt = ph1.tile([128, D], F32, name="xld")
                nc.sync.dma_start(xt[:], x_io[tb * 128:(tb + 1) * 128, :])
                ssq = ph1.tile([128, 1], F32, name="nrm_ssq")
                scr = ph1.tile([128, D], BF, name="nrm_scr")
                nc.scalar.activation(scr[:], xt[:], AF.Square,
                                     accum_out=ssq[:])
                # sd = sqrt(ssq/(16 D) + eps/16) = rms/4 ;  inv = 4/rms
                sd = ph1.tile([128, 1], F32, name="nrm_sd")
                nc.scalar.activation(sd[:], ssq[:], AF.Sqrt,
                                     bias=eps16_t[:],
                                     scale=1.0 / (D * SX * SX))
                inv = ph1.tile([128, 1], F32, name="nrm_inv")
                nc.vector.reciprocal(inv[:], sd[:])
                xn4 = ph1.tile([128, D], BF, name="nrm_xn4")
                nc.scalar.activation(xn4[:], xt[:], AF.Copy, scale=inv[:])
                for dc in range(DC):
                    tp = ps_tp1.tile([128, 128], BF, name="tp1")
                    nc.tensor.transpose(tp[:], xn4[:, dc * 128:(dc + 1) * 128],
                                        ident[:])
                    nc.vector.tensor_copy(xzt8[:, dc, tb * 128:(tb + 1) * 128],
                                          tp[:])
            nc.sync.dma_start(ones[:], ones_io[:, :])
            nc.sync.dma_start(ones8[:], ones8_io[:, :, :])
            nc.sync.dma_start(mask_sb[:], masks_io[:])

        # ============== phase 2: qkv projections + AllGather ==============
        # order: K (16 heads) -> AG_K ; V -> AG_V ; Q (overlaps the AGs)
        with tc.tile_pool(name="wqk", bufs=2) as wqk, \
             tc.tile_pool(name="wv", bufs=2) as wvp, \
             tc.tile_pool(name="ph2", bufs=1) as ph2, \
             tc.tile_pool(name="ps_mm", bufs=3, space="PSUM") as ps_mm:
            kt_l = ph2.tile([128, H, T], F8, name="kt_l")
            v_sb = ph2.tile([128, TB, D], F8, name="v_sb")

            def qk_head(idx, dst, h):
                wt = wqk.tile([128, DP, 2, 128], F8, name="wqk_t")
                nc.sync.dma_start(wt[:], wqk_io[idx, :, :, :, :])
                ps = ps_mm.tile([128, T], F32, name="ps_qk")
                for p in range(DP):
                    nc.tensor.matmul(ps[:], wt[:, p, :, :],
                                     xzt8[:, 2 * p:2 * p + 2, :],
                                     start=(p == 0), stop=(p == DP - 1),
                                     perf_mode=DR)
                nc.vector.tensor_scalar_mul(dst[:, h, :], ps[:], c4i[:])

            for h in range(H):  # K
                qk_head(h, kt_l, h)
            for h in range(H):
                nc.sync.dma_start(kt_c[h * 128:(h + 1) * 128, :],
                                  kt_l[:, h, :])
            if cfg.fake_ag:
                nc.sync.dma_start(kt_g[0:D, :], kt_c[:, :])
            else:
                nc.gpsimd.collective_compute(
                    "AllGather", mybir.AluOpType.bypass,
                    replica_groups=[list(range(n))],
                    ins=[kt_c.opt()], outs=[kt_g.opt()])

            for g in range(D // 512):  # V (flipped operands; token-major out)
                wt = wvp.tile([128, DP, 2, 512], F8, name="wv_t")
                nc.sync.dma_start(wt[:], wv_io[g, :, :, :, :])
                for tb in range(TB):
                    ps = ps_mm.tile([128, 512], F32, name="ps_v")
                    for p in range(DP):
                        nc.tensor.matmul(
                            ps[:],
                            xzt8[:, 2 * p:2 * p + 2, tb * 128:(tb + 1) * 128],
                            wt[:, p, :, :],
                            start=(p == 0), stop=(p == DP - 1), perf_mode=DR)
                    nc.vector.tensor_scalar_mul(
                        v_sb[:, tb, g * 512:(g + 1) * 512], ps[:], c16i[:])
            for tb in range(TB):
                nc.sync.dma_start(v_c[tb * 128:(tb + 1) * 128, :],
                                  v_sb[:, tb, :])
            if cfg.fake_ag:
                nc.sync.dma_start(v_g[0:T, :], v_c[:, :])
            else:
                nc.gpsimd.collective_compute(
                    "AllGather", mybir.AluOpType.bypass,
                    replica_groups=[list(range(n))],
                    ins=[v_c.opt()], outs=[v_g.opt()])

            for h in range(H):  # Q (overlaps the AGs)
                qk_head(H + h, qt, h)

        # -- prefetch first FFN-up weight groups during attention --
        w13p = top.enter_context(tc.tile_pool(name="w13", bufs=2))
        w13p8 = top.enter_context(tc.tile_pool(name="w13f8", bufs=2))

        def load_w13(g):
            t = w13p.tile([128, DC, 512], BF, name="w13_t")
            nc.sync.dma_start(t[:], w13_io[g, :, :, :])
            return t

        def load_w13f8(g):
            t = w13p8.tile([128, 4, DP, 2, 128], F8, name="w13f8_t")
            nc.sync.dma_start(t[:], w13f8_io[g, :, :, :, :, :])
            return t

        w13_pre = {g: load_w13(g) for g in (8, 9)}
        w13f8_pre = {g: load_w13f8(g) for g in (0, 1)}
        w2p = top.enter_context(tc.tile_pool(name="w2p", bufs=2))
        w2_pre = {}

        def load_w2(gf, c0, pw):
            t = w2p.tile([128, W2G, pw], BF, name="w2_t")
            nc.sync.dma_start(t[:], w2_io[gf, :, :, c0:c0 + pw])
            return t

        # ================= phase 4: attention =================
        # Strided token assignment: core c owns tokens == c (mod n).
        # Chunk pairs (ci = rank pair) are processed as [128, 2, 2, 128]
        # (ci, batch, q) tiles; exp -> fp8; AV + denominator via DoubleRow.
        HG = min(4, H)
        with tc.tile_pool(name="kv", bufs=2) as kv, \
             tc.tile_pool(name="ktp", bufs=3) as ktp, \
             tc.tile_pool(name="esb", bufs=8) as esb, \
             tc.tile_pool(name="aux", bufs=3) as aux, \
             tc.tile_pool(name="ps_s", bufs=3, space="PSUM") as ps_s, \
             tc.tile_pool(name="ps_ot", bufs=2, space="PSUM") as ps_ot, \
             tc.tile_pool(name="ps_r", bufs=1, space="PSUM") as ps_r:
            for hq in range(H // HG):
                vtb = [kv.tile([128, SB2 * NP2, 2, HG * 128], F8,
                               name=f"vtb{qb}") for qb in range(2)]
                for qb in range(2):
                    for j2 in range(SB2):
                        src_ap = v_g4[:, :, j2 * 256 + qb * 128:
                                      j2 * 256 + qb * 128 + 128,
                                      hq * HG * 128:(hq + 1) * HG * 128]
                        nc.sync.dma_start(
                            vtb[qb][:, j2 * NP2:(j2 + 1) * NP2, :, :],
                            src_ap.rearrange("a b p c -> p a b c"))
                for hi in range(HG):
                    h = hq * HG + hi
                    ktb = ktp.tile([128, n, T], F8, name="ktb")
                    nc.sync.dma_start(
                        ktb[:, :, :],
                        kt_g3[:, h * 128:(h + 1) * 128, :].rearrange(
                            "r p t -> p r t"))
                    for b in range(SB2):
                        otp = [ps_ot.tile([128, 128], F32, name=f"otp{qb}")
                               for qb in range(2)]
                        rp = ps_r.tile([128, 256], F32, name="rp")
                        npairs = (b + 1) * NP2
                        pi = 0
                        for j2 in range(b + 1):
                            for pm in range(NP2):
                                sp = ps_s.tile([128, 2, 2, 128], F32,
                                               name="sp")
                                for ci in range(2):
                                    r = 2 * pm + ci
                                    for qb in range(2):
                                        nc.tensor.matmul(
                                            sp[:, ci, qb, :],
                                            ktb[:, r,
                                                j2 * 256 + qb * 128:
                                                j2 * 256 + (qb + 1) * 128],
                                            qt[:, h,
                                               b * 256 + qb * 128:
                                               b * 256 + (qb + 1) * 128],
                                            start=True, stop=True)
                                e = esb.tile([128, 2, 2, 128], F8, name="e")
                                nc.scalar.activation(e[:], sp[:], AF.Exp,
                                                     bias=nebias_t[:],
                                                     scale=1.0 / 256)
                                if j2 == b:
                                    nc.vector.tensor_mul(
                                        e[:], e[:], mask_sb[:, pm])
                                nc.tensor.matmul(
                                    rp[:], ones8[:, :, :], e[:],
                                    start=(pi == 0),
                                    stop=(pi == npairs - 1), perf_mode=DR)
                                for qb in range(2):
                                    nc.tensor.matmul(
                                        otp[qb][:],
                                        vtb[qb][:, j2 * NP2 + pm, :,
                                                hi * 128:(hi + 1) * 128],
                                        e[:, :, qb, :],
                                        start=(pi == 0),
                                        stop=(pi == npairs - 1),
                                        perf_mode=DR)
                                pi += 1
                        rinv = aux.tile([128, 256], F32, name="rinv")
                        nc.vector.reciprocal(rinv[:], rp[:])
                        for qb in range(2):
                            nc.vector.tensor_mul(
                                ot8[:, h,
                                    b * 256 + qb * 128:b * 256 + (qb + 1) * 128],
                                otp[qb][:], rinv[:, qb * 128:(qb + 1) * 128])

        # ============== phase 5: out-proj (fp8 DR) + residual ==============
        with tc.tile_pool(name="ph5", bufs=2) as ph5, \
             tc.tile_pool(name="ps_y", bufs=2, space="PSUM") as ps_y:
            for ct in range(CT):
                c0 = ct * CW
                wo_g = ph5.tile([128, H // 2, 2, CW], F8, name="wo_g")
                nc.sync.dma_start(wo_g[:], wout_io[ct, :, :, :, :])
                for tb in range(TB):
                    ps = ps_y.tile([128, CW], F32, name="ps_y")
                    for hp in range(H // 2):
                        nc.tensor.matmul(
                            ps[:],
                            ot8[:, 2 * hp:2 * hp + 2, tb * 128:(tb + 1) * 128],
                            wo_g[:, hp, :, :],
                            start=(hp == 0), stop=(hp == H // 2 - 1),
                            perf_mode=DR)
                    yo = ph5.tile([128, CW], F32, name="yo")
                    nc.vector.tensor_scalar_mul(yo[:], ps[:], c64i[:])
                    xr = ph5.tile([128, CW], F32, name="xr")
                    nc.sync.dma_start(
                        xr[:], x_io[tb * 128:(tb + 1) * 128, c0:c0 + CW])
                    nc.vector.tensor_add(x1_t[tb][:, c0:c0 + CW], yo[:],
                                         xr[:])

        # ============== phase 6: rmsnorm2 + transpose (bf16) ==============
        with tc.tile_pool(name="ph6", bufs=2) as ph6, \
             tc.tile_pool(name="ps_tp6", bufs=4, space="PSUM") as ps_tp6:
            for tb in range(TB):
                xt = x1_t[tb]
                ssq = ph6.tile([128, 1], F32, name="n2_ssq")
                scr = ph6.tile([128, D], BF, name="n2_scr")
                nc.scalar.activation(scr[:], xt[:], AF.Square,
                                     accum_out=ssq[:])
                sd = ph6.tile([128, 1], F32, name="n2_sd")
                nc.scalar.activation(sd[:], ssq[:], AF.Sqrt,
                                     bias=eps_t[:], scale=1.0 / D)
                inv = ph6.tile([128, 1], F32, name="n2_inv")
                nc.vector.reciprocal(inv[:], sd[:])
                xn = ph6.tile([128, D], BF, name="n2_xn")
                nc.scalar.activation(xn[:], xt[:], AF.Copy, scale=inv[:])
                for dc in range(DC):
                    tp = ps_tp6.tile([128, 128], BF, name="tp")
                    nc.tensor.transpose(tp[:], xn[:, dc * 128:(dc + 1) * 128],
                                        ident[:])
                    nc.vector.tensor_copy(xzt2[:, dc, tb * 128:(tb + 1) * 128],
                                          tp[:])
                    nc.vector.tensor_scalar_mul(
                        xzt2f8[:, dc, tb * 128:(tb + 1) * 128], tp[:], c4[:])

        # ============== phase 7: FFN up (w1/w3 + swiglu) ==============
        with tc.tile_pool(name="ph7", bufs=3) as ph7, \
             tc.tile_pool(name="ps_h", bufs=3, space="PSUM") as ps_h:
            w2_pre[(0, 0)] = load_w2(0, 0, 2 * CW)
            for g in range(FFB // 2):  # 512-col groups (2 ff blocks)
                if g < 8:  # fp8 DoubleRow quarter (psum = 64x true)
                    wt8 = (w13f8_pre.pop(g) if g in w13f8_pre
                           else load_w13f8(g))
                    for fi in range(2):
                        f = 2 * g + fi
                        h1 = ps_h.tile([128, T], F32, name="h1")
                        for p in range(DP):
                            nc.tensor.matmul(
                                h1[:], wt8[:, 2 * fi, p, :, :],
                                xzt2f8[:, 2 * p:2 * p + 2, :],
                                start=(p == 0), stop=(p == DP - 1),
                                perf_mode=DR)
                        s1 = ph7.tile([128, T], BF, name="s1")
                        nc.scalar.activation(s1[:], h1[:], AF.Silu,
                                             scale=1.0 / 64)
                        h3 = ps_h.tile([128, T], F32, name="h3")
                        for p in range(DP):
                            nc.tensor.matmul(
                                h3[:], wt8[:, 2 * fi + 1, p, :, :],
                                xzt2f8[:, 2 * p:2 * p + 2, :],
                                start=(p == 0), stop=(p == DP - 1),
                                perf_mode=DR)
                        hh = ph7.tile([128, T], BF, name="hh")
                        nc.vector.scalar_tensor_tensor(
                            hh[:], h3[:], 1.0 / 64, s1[:],
                            mybir.AluOpType.mult, mybir.AluOpType.mult)
                        nc.sync.dma_start(hh_d[f * 128:(f + 1) * 128, :],
                                          hh[:])
                    continue
                wt = w13_pre.pop(g) if g in w13_pre else load_w13(g)
                for fi in range(2):
                    f = 2 * g + fi
                    o1, o3 = fi * 256, fi * 256 + 128
                    h1 = ps_h.tile([128, T], F32, name="h1")
                    for dc in range(DC):
                        nc.tensor.matmul(h1[:], wt[:, dc, o1:o1 + 128],
                                         xzt2[:, dc, :],
                                         start=(dc == 0), stop=(dc == DC - 1))
                    s1 = ph7.tile([128, T], BF, name="s1")
                    if cfg.use_silu:
                        nc.scalar.activation(s1[:], h1[:], AF.Silu)
                    else:
                        sg = ph7.tile([128, T], BF, name="sg")
                        nc.scalar.activation(sg[:], h1[:], AF.Sigmoid)
                        nc.vector.tensor_mul(s1[:], sg[:], h1[:])
                    h3 = ps_h.tile([128, T], F32, name="h3")
                    for dc in range(DC):
                        nc.tensor.matmul(h3[:], wt[:, dc, o3:o3 + 128],
                                         xzt2[:, dc, :],
                                         start=(dc == 0), stop=(dc == DC - 1))
                    hh = ph7.tile([128, T], BF, name="hh")
                    nc.vector.tensor_mul(hh[:], s1[:], h3[:])
                    nc.sync.dma_start(hh_d[f * 128:(f + 1) * 128, :], hh[:])

        # ============== phase 8: FFN down + residual + out ==============
        with tc.tile_pool(name="hhp", bufs=4) as hhp, \
             tc.tile_pool(name="ps_y2", bufs=1, space="PSUM") as ps_y2, \
             tc.tile_pool(name="osb", bufs=2) as osb:
            per_pass = max(1, 8 // TB)  # col tiles per pass (8 psum banks)
            for p0 in range(0, CT, per_pass):
                cts = list(range(p0, min(CT, p0 + per_pass)))
                pw = len(cts) * CW
                ps_t = {(tb, ct): ps_y2.tile([128, CW], F32,
                                             name=f"y2_{tb}_{ct - p0}")
                        for tb in range(TB) for ct in cts}
                for gf in range(FFB // W2G):
                    wt = (w2_pre.pop((gf, p0))
                          if (gf, p0) in w2_pre
                          else load_w2(gf, p0 * CW, pw))
                    for fi in range(W2G):
                        fc = gf * W2G + fi
                        hh = hhp.tile([128, T], BF, name="hh_s")
                        nc.sync.dma_start(hh[:],
                                          hh_d[fc * 128:(fc + 1) * 128, :])
                        for tb in range(TB):
                            for ct in cts:
                                o = (ct - p0) * CW
                                nc.tensor.matmul(
                                    ps_t[(tb, ct)][:],
                                    hh[:, tb * 128:(tb + 1) * 128],
                                    wt[:, fi, o:o + CW],
                                    start=(fc == 0), stop=(fc == FFB - 1))
                for tb in range(TB):
                    for ct in cts:
                        c0 = ct * CW
                        o = osb.tile([128, CW], F32, name="o_sb")
                        nc.vector.tensor_add(o[:], ps_t[(tb, ct)][:],
                                             x1_t[tb][:, c0:c0 + CW])
                        nc.sync.dma_start(
                            out_io[tb * 128:(tb + 1) * 128, c0:c0 + CW], o[:])

    nc.compile()
    return nc


# --------------------------- host-side prep ---------------------------

def host_prep(cfg: Cfg, x, w_qkv, w_out, w1, w2, w3, g1, g2):
    """Build the per-core input maps (numpy, fp8/bf16 weights, mask data)."""
    n, D, H, DFF = cfg.n_cores, cfg.D, cfg.H, cfg.DFF
    DC, DP, FFB, T, NP2 = cfg.DC, cfg.DP, cfg.FFB, cfg.T, cfg.NP2
    CT = cfg.CT

    x = np.asarray(x, np.float32)
    g1 = np.asarray(g1, np.float32)
    g2 = np.asarray(g2, np.float32)

    wqkv = np.asarray(w_qkv, np.float32) * g1[:, None]
    wqkv = wqkv.copy()
    wqkv[:, :D] *= cfg.Hd ** -0.5  # fold softmax scale into W_q

    # Q/K blocks for DR: [2H, 128, DP, 2, 128]; K heads (cols D..2D) first
    def qk_blocks(w_cols):  # w_cols [D, H*128]
        # [kt(16),krow(128),h,col] -> [h, krow, pair, two, col]
        a = (w_cols * SW).reshape(DP, 2, 128, H, 128)
        return np.ascontiguousarray(a.transpose(3, 2, 0, 1, 4)).astype(E4M3)

    wqk8 = np.concatenate(
        [qk_blocks(wqkv[:, D:2 * D]), qk_blocks(wqkv[:, 0:D])], axis=0)

    # V as DR moving operand: [4, 128, DP, 2, 512]
    wv = (wqkv[:, 2 * D:3 * D] * SW).reshape(DP, 2, 128, 4, 512)
    wv8 = np.ascontiguousarray(wv.transpose(3, 2, 0, 1, 4)).astype(E4M3)

    # out-proj: [CT, 128, H/2, 2, CW]
    wo = (np.asarray(w_out, np.float32) * SW).reshape(H // 2, 2, 128, CT,
                                                      cfg.CW)
    wout8 = np.ascontiguousarray(wo.transpose(3, 2, 0, 1, 4)).astype(E4M3)

    def group_layout(w, gw):
        C = w.shape[1]
        return np.ascontiguousarray(
            w.reshape(DC, 128, C // gw, gw).transpose(2, 1, 0, 3))

    DP_h = DC // 2
    w1g = (np.asarray(w1, np.float32) * g2[:, None]).reshape(DC, 128, FFB, 128)
    w3g = (np.asarray(w3, np.float32) * g2[:, None]).reshape(DC, 128, FFB, 128)
    w13 = np.stack([w1g, w3g], axis=3).reshape(DC, 128, 2 * DFF)
    w13flat = w13.reshape(DC * 128, 2 * DFF)
    w13_t = group_layout(w13flat, 512).astype(BF16)
    # fp8 quarter (interleaved cols 0..4095 = w1/w3 for ff 0..2047), 16x
    a8 = (w13flat[:, :4096] * SW).reshape(DP_h, 2, 128, 32, 128)
    w13f8 = np.ascontiguousarray(
        a8.transpose(3, 2, 0, 1, 4).reshape(8, 4, 128, DP_h, 2, 128)
        .transpose(0, 2, 1, 3, 4, 5)).astype(E4M3)

    w2_t = np.ascontiguousarray(
        np.asarray(w2, np.float32).reshape(FFB // cfg.W2G, cfg.W2G, 128, D)
        .transpose(0, 2, 1, 3)).astype(BF16)

    ident = np.eye(128, dtype=np.float32)
    ones128 = np.ones((128, 128), np.float32).astype(BF16)
    ones8 = np.ones((128, 2, 128), np.float32).astype(E4M3)

    # local order: l = blk*256 + batch*128 + i ; token = n*(blk*128+i) + c
    l = np.arange(cfg.T)
    blk, qb_a, i_a = l // 256, (l // 128) % 2, l % 128
    base_pos = n * (blk * 128 + i_a)

    in_maps = []
    kp = np.arange(128)[:, None]
    qq = np.arange(128)[None, :]
    for c in range(n):
        pos = base_pos + c
        x_c = np.ascontiguousarray(x[qb_a, pos, :])
        # 0/1 masks for diagonal chunks, applied to e post-exp
        # allowed iff kp < q or (kp == q and r <= c);  [128, NP2, 2, 2, 128]
        masks = np.zeros((128, NP2, 2, 2, 128), np.float32)
        for pm in range(NP2):
            for ci in range(2):
                r = 2 * pm + ci
                m = ((kp < qq) | ((kp == qq) & (r <= c))).astype(np.float32)
                masks[:, pm, ci, 0, :] = m
                masks[:, pm, ci, 1, :] = m
        in_maps.append({
            "x_c": x_c,
            "wqk8": wqk8, "wv8": wv8, "wout8": wout8,
            "w13_t": w13_t, "w2_t": w2_t, "w13f8": w13f8,
            "masks": masks.astype(E4M3),
            "ident": ident.astype(BF16), "ident8": ident.astype(E4M3),
            "ones128": ones128, "ones8": ones8,
        })
    return in_maps


def assemble(cfg: Cfg, results):
    n = cfg.n_cores
    out = np.empty((cfg.B, cfg.S, cfg.D), np.float32)
    l = np.arange(cfg.T)
    blk, qb_a, i_a = l // 256, (l // 128) % 2, l % 128
    base_pos = n * (blk * 128 + i_a)
    for c in range(n):
        out[qb_a, base_pos + c] = results[c]["out_c"]
    return out


_NC_CACHE = {}


def get_nc(cfg: Cfg = FULL):
    if cfg not in _NC_CACHE:
        _NC_CACHE[cfg] = build_nc(cfg)
    return _NC_CACHE[cfg]


def kernel(x, w_qkv, w_out, w1, w2, w3, g1, g2):
    cfg = FULL
    nc = get_nc(cfg)
    in_maps = host_prep(cfg, x, w_qkv, w_out, w1, w2, w3, g1, g2)
    res = run_bass_kernel_spmd(nc, in_maps, core_ids=list(range(cfg.n_cores)))
    return assemble(cfg, res.results)

